# revision 48
# baseline (speedup 1.0000x reference)
"""Trainium2 Bass kernel for Physics-Attention over an irregular mesh (v2).

Contract: kernel(**inputs) takes the FULL inputs from setup_inputs() and
returns the FULL [4, 32768, 256] f32 output, distributing across 8 cores
internally (one (batch, half-of-N) shard per core, pairwise AllReduce on the
slice-token pooling reductions).

v2 vs baseline (494us -> ~280us):
 - sw transposes via DMA xbar transpose (one 4-tile dma_start_transpose on
   the sync queue, one-group lag) instead of PE transposes + PSUM->SBUF copy
 - head-paired pooling matmuls (4 instead of 8 per tile), emitted with a
   3-tile lag so the strict-FIFO PE queue never stalls on the softmax chain
 - input loads as 16-tile batches on the sync queue (big batches keep DMA
   semaphore recycling off the critical path), prefetched 2 batches ahead
 - engine rebalance in pass 1: exp on Act, reduce/recip/fxs-half on DVE,
   fxs-half on Act, normalize-multiply on GpSimd
 - bf16 AllReduce payload in head-pair layout
 - head-pair-packed slice-attention stage: rsqrt = exp(-0.5*ln(n2)),
   softmax denominator fused into the av matmul via a ones column on v
 - pass 2 transposed: C chunks stationary, 4 tiles of swT as one 512-wide
   moving operand, output written out^T (host untransposes), oT PSUM tiles
   rotated over 3 tags so matmuls never wait on the output copies
"""

import sys

sys.path.insert(0, "/opt/trn_rl_repo")

import numpy as np
import ml_dtypes

import concourse.bass as bass
import concourse.mybir as mybir
import concourse.tile as tile
from concourse import bacc, bass_utils
from concourse.bass import ts

F32 = mybir.dt.float32
BF16 = mybir.dt.bfloat16
AF = mybir.ActivationFunctionType
ALU = mybir.AluOpType

B, N, DIM = 4, 32768, 256
H, D, G = 8, 64, 64
INNER = H * D  # 512
NCORES = 8
NLOC = N // 2          # 16384 tokens per core
TOK = 128              # tokens per tile
T = NLOC // TOK        # 128 tiles
KCH = DIM // 128       # 2 contraction chunks
EPS_SLICE = 1e-5

_CACHE = {}


def _build(attn_scale: float, res_scale: float):
    """Build the single-core SPMD program (identical on all 8 cores)."""
    nc = bacc.Bacc("TRN2", target_bir_lowering=False, debug=False,
                   enable_asserts=False, num_devices=NCORES)

    xT_d = nc.dram_tensor("xT", [DIM, NLOC], BF16, kind="ExternalInput").ap()
    # combined moving weights: cols 0:512 = A^T, 512:1024 = Wfx^T
    AWT_d = nc.dram_tensor("AWT", [DIM, 2 * INNER], BF16, kind="ExternalInput").ap()
    idbf_d = nc.dram_tensor("idbf", [128, 128], BF16, kind="ExternalInput").ap()
    WqT_d = nc.dram_tensor("WqT", [D, D], BF16, kind="ExternalInput").ap()
    WkT_d = nc.dram_tensor("WkT", [D, D], BF16, kind="ExternalInput").ap()
    WvT_d = nc.dram_tensor("WvT", [D, D], BF16, kind="ExternalInput").ap()
    WoT_d = nc.dram_tensor("WoT", [INNER, DIM], BF16, kind="ExternalInput").ap()
    # output stored TRANSPOSED: out_d[fh, fl, tok] = out[tok, 128*fh + fl]
    out_d = nc.dram_tensor("out", [2, 128, NLOC], BF16,
                           kind="ExternalOutput").ap()

    xT_v = xT_d.rearrange("(c p) n -> p c n", p=128)      # [128, 2, NLOC]
    AWT_v = AWT_d.rearrange("(c p) n -> p c n", p=128)    # [128, 2, 1024]
    WoT_v = WoT_d.rearrange("(h d) f -> d h f", d=64)     # [64, 8, 256]

    with tile.TileContext(nc) as tc:
        with (
            tc.tile_pool(name="consts", bufs=1) as consts,
            tc.tile_pool(name="store", bufs=1) as store,
            tc.tile_pool(name="work", bufs=4) as work,
            tc.tile_pool(name="xtp", bufs=3) as xtp,
            tc.tile_pool(name="swnp", bufs=5) as swnp,
            tc.tile_pool(name="obuf", bufs=2) as obufp,
            tc.tile_pool(name="small", bufs=3) as small,
            tc.tile_pool(name="stage", bufs=1) as stg_pool,
            tc.tile_pool(name="psmm", bufs=2, space="PSUM") as psmm,
            tc.tile_pool(name="psacc", bufs=1, space="PSUM") as psacc,
            tc.tile_pool(name="dram", bufs=1, space="DRAM") as dram,
        ):
            # (no warmup collective: the gpsimd trigger spin-waits for the CC
            # and would block the pass-1 normalize multiplies for ~35us)

            # ---- resident constants (AWT first: first matmul needs it) ----
            AWT_sb = consts.tile([128, KCH, 2 * INNER], BF16)
            nc.sync.dma_start(AWT_sb, AWT_v)
            # input loads in 16-tile batches on the Act HWDGE queue —
            # separate from the sw transposes (sync queue), and batches big
            # enough that DMA-semaphore reuse never creates false waits
            xt16_grps = {}
            for g in range(2):
                xt16_grps[g] = xtp.tile([128, KCH, 16 * TOK], BF16,
                                        tag="xt16", name=f"xt16_{g}")
                nc.sync.dma_start(
                    xt16_grps[g],
                    xT_v[:, :, 16 * g * TOK:16 * (g + 1) * TOK])
            idbf = consts.tile([128, 128], BF16)
            nc.sync.dma_start(idbf, idbf_d)
            WqT_sb = consts.tile([64, 64], BF16)
            nc.sync.dma_start(WqT_sb, WqT_d)
            WkT_sb = consts.tile([64, 64], BF16)
            nc.sync.dma_start(WkT_sb, WkT_d)
            WvT_sb = consts.tile([64, 64], BF16)
            nc.sync.dma_start(WvT_sb, WvT_d)
            WoT_sb = consts.tile([64, H, DIM], BF16)
            nc.sync.dma_start(WoT_sb, WoT_v)

            # Pin the Act table to the set containing ln+exp+copy+square: a
            # dummy ln as the very first Act instruction makes lower_act pick
            # natural_log_exp_and_others for the initial load, so the stage's
            # rsqrt (exp(-0.5*ln(n2))) triggers no mid-kernel table swap.
            actw = consts.tile([1, 4], F32, name="actw")
            nc.gpsimd.memset(actw, 1.0)
            actw2 = consts.tile([1, 4], F32, name="actw2")
            nc.scalar.activation(actw2, actw, AF.Ln)

            # transposed normalized routing weights, [128, tile, chunk, tok]
            swT_store = store.tile([128, T, 4, TOK], BF16)

            # slice-token accumulators in PAIR layout:
            # st_ps[i][p, jj, par, 0:65] with j = 2i+jj covering heads
            # (2j, 2j+1); valid data at par == p//64:
            #   head 2j+par, g = p%64; col 64 = snorm
            st_ps = [psacc.tile([128, 2, 2, D + 1], F32, name=f"st_ps{i}")
                     for i in range(2)]

            # fxs ring buffers: pre-fill the ones column once (reused by the
            # ring; per-tile cast only writes cols 0:64)
            fxs_bufs = [work.tile([128, H, D + 1], BF16, tag="fxs",
                                  name=f"fxs{i}")
                        for i in range(4)]
            for fb in fxs_bufs:
                nc.gpsimd.memset(fb[:, :, D], 1.0)

            # normalized sw in 4-tile groups (feeds pooling + DMA transpose)
            def swn_slot(tt):
                grp = swnp.tile([128, 4, INNER], BF16, tag="swn4",
                                name=f"swn4_{tt // 4}")
                return grp

            swn_grps = {}

            # PE order is strict FIFO for matmuls, so pool matmuls run with a
            # 3-tile lag: tile t's mains are never queued behind pool matmuls
            # waiting on the softmax chain (~3us exp->reduce->recip->mult).
            LAG = 3

            def emit_pool(tt):
                grp = swn_grps[tt // 4]
                sw = grp[:, tt % 4, :]
                fxs = fxs_bufs[tt % 4]
                for j in range(4):
                    nc.tensor.matmul(
                        st_ps[j // 2][:, j % 2, :, :].rearrange("p a b -> p (a b)"),
                        sw[:, ts(j, 128)],
                        fxs[:, 2 * j:2 * j + 2, :].rearrange("p a b -> p (a b)"),
                        start=(tt == 0 and j % 2 == 0),
                        stop=(tt == T - 1 and j % 2 == 1))

            # ================= PASS 1 =================
            xt4 = None
            for t in range(T):
                if t % 16 == 0:
                    gp = t // 16 + 2
                    if gp < T // 16:
                        xt16_grps[gp] = xtp.tile(
                            [128, KCH, 16 * TOK], BF16, tag="xt16",
                            name=f"xt16_{gp}")
                        nc.sync.dma_start(
                            xt16_grps[gp],
                            xT_v[:, :, 16 * gp * TOK:16 * (gp + 1) * TOK])
                    xt16 = xt16_grps.pop(t // 16)
                xt = xt16[:, :, (t % 16) * TOK:(t % 16 + 1) * TOK]

                lg = psmm.tile([128, H, G], F32, tag="lg")
                fxp = psmm.tile([128, H, D], F32, tag="fx")
                # lg finishes first so the softmax chain starts earlier
                for k in range(KCH):
                    nc.tensor.matmul(lg, xt[:, k, :], AWT_sb[:, k, 0:INNER],
                                     start=(k == 0), stop=(k == KCH - 1))
                for k in range(KCH):
                    nc.tensor.matmul(fxp, xt[:, k, :], AWT_sb[:, k, INNER:],
                                     start=(k == 0), stop=(k == KCH - 1))

                if t >= LAG:
                    emit_pool(t - LAG)

                usw = work.tile([128, H, G], BF16, tag="usw")
                nc.scalar.activation(usw, lg, AF.Exp)
                den = small.tile([128, H], BF16, tag="den")
                with nc.allow_low_precision(reason="softmax denom in bf16"):
                    nc.vector.reduce_sum(den, usw, axis=mybir.AxisListType.X)
                rden = small.tile([128, H], F32, tag="rden")
                nc.vector.reciprocal(rden, den)
                if t % 4 == 0:
                    swn_grps[t // 4] = swn_slot(t)
                swn = swn_grps[t // 4][:, t % 4, :].rearrange(
                    "p (h g) -> p h g", h=H)
                nc.gpsimd.tensor_tensor(
                    swn, usw, rden[:, :, None].to_broadcast([128, H, G]),
                    ALU.mult)

                fxs = fxs_bufs[t % 4]
                nc.scalar.copy(fxs[:, 0:4, 0:D], fxp[:, 0:4, :])
                nc.vector.tensor_copy(fxs[:, 4:8, 0:D], fxp[:, 4:8, :])

                if t % 4 == 3 and t >= 4:
                    # one xbar transpose for 4 tiles of normalized sw, with
                    # a one-group lag so the in-order sync queue never waits
                    # on the softmax chain (input loads are on the Act queue)
                    g4 = t // 4 - 1
                    nc.sync.dma_start_transpose(
                        swT_store[:, 4 * g4:4 * g4 + 4].rearrange(
                            "p t c j -> p (t c) j"),
                        swn_grps[g4].rearrange("p t n -> p (t n)"))

            for tt in range(T - LAG, T):
                emit_pool(tt)
            g4 = T // 4 - 1
            nc.sync.dma_start_transpose(
                swT_store[:, 4 * g4:4 * g4 + 4].rearrange(
                    "p t c j -> p (t c) j"),
                swn_grps[g4].rearrange("p t n -> p (t n)"))

            # ================= AllReduce (bf16, pair layout) =================
            ccbuf = stg_pool.tile([128, 4, D + 1], BF16)
            # select the valid parity per partition half (no partition shift)
            nc.vector.tensor_copy(ccbuf[0:64, 0:2, :], st_ps[0][0:64, :, 0, :])
            nc.scalar.copy(ccbuf[64:128, 0:2, :], st_ps[0][64:128, :, 1, :])
            nc.vector.tensor_copy(ccbuf[0:64, 2:4, :], st_ps[1][0:64, :, 0, :])
            nc.scalar.copy(ccbuf[64:128, 2:4, :], st_ps[1][64:128, :, 1, :])

            cc_in = dram.tile([128, 4 * (D + 1)], BF16)
            cc_out = dram.tile([128, 4 * (D + 1)], BF16)
            # issue from the Act queue: the sync queue is still busy with the
            # tail DMA transposes, which pass 2 doesn't need for a while
            nc.scalar.dma_start(cc_in, ccbuf.rearrange("p a b -> p (a b)"))
            nc.gpsimd.collective_compute(
                "AllReduce", ALU.add,
                replica_groups=[[0, 1], [2, 3], [4, 5], [6, 7]],
                ins=[cc_in.opt()], outs=[cc_out.opt()],
            )
            stg = stg_pool.tile([128, 4, D + 1], BF16)
            nc.sync.dma_start(stg.rearrange("p a b -> p (a b)"), cc_out)

            # ============ stage: slice attention in pair layout ============
            # partition p: g = p%64, head parity p//64; slot j: heads (2j,2j+1)
            snorm_e = stg_pool.tile([128, 4], F32)
            nc.vector.tensor_scalar_add(snorm_e, stg[:, :, D], EPS_SLICE)
            rs = stg_pool.tile([128, 4], F32)
            nc.vector.reciprocal(rs, snorm_e)
            st_sb = stg_pool.tile([128, 4, D], BF16)
            nc.vector.tensor_tensor(st_sb, stg[:, :, 0:D],
                                    rs[:, :, None].to_broadcast([128, 4, D]),
                                    ALU.mult)

            # kv = sum over heads of st (the /H is folded into WkT/WvT)
            kvp = stg_pool.tile([128, D], BF16)
            with nc.allow_low_precision(reason="kv mean of 4 bf16 values"):
                nc.vector.reduce_sum(kvp, st_sb.rearrange("p j d -> p d j"),
                                     axis=mybir.AxisListType.X)
            kvpT_ps = psmm.tile([64, 128], BF16, tag="op")
            nc.tensor.transpose(kvpT_ps, kvp, idbf)
            kvpT_sb = stg_pool.tile([64, 128], BF16)
            nc.vector.tensor_copy(kvpT_sb, kvpT_ps)
            kvT_sb = stg_pool.tile([64, D], BF16)
            nc.vector.tensor_tensor(kvT_sb, kvpT_sb[:, 0:64],
                                    kvpT_sb[:, 64:128], ALU.add)

            # stT: 4 pair transposes
            stT_ps = psmm.tile([64, 4, 128], BF16, tag="op")
            for j in range(4):
                nc.tensor.transpose(stT_ps[:, j, :], st_sb[:, j, :], idbf)
            stT_sb = stg_pool.tile([64, 4, 128], BF16)
            nc.scalar.copy(stT_sb, stT_ps)

            # q per pair; k/v from kv
            q_ps = psmm.tile([128, 4, D], F32, tag="lg")
            for j in range(4):
                nc.tensor.matmul(q_ps[:, j, :], stT_sb[:, j, :], WqT_sb,
                                 start=(j == 0), stop=(j == 3))
            kv_ps = psmm.tile([64, 2, D], F32, tag="op")
            nc.tensor.matmul(kv_ps[:, 0, :], kvT_sb, WkT_sb,
                             start=True, stop=False)
            nc.tensor.matmul(kv_ps[:, 1, :], kvT_sb, WvT_sb,
                             start=False, stop=True)
            # v with a ones column: the av matmul then also yields the
            # attention softmax denominator (sum over s) in column D
            v_sb = stg_pool.tile([64, D + 1], BF16)
            nc.vector.tensor_copy(v_sb[:, 0:D], kv_ps[:, 1, :])
            nc.gpsimd.memset(v_sb[:, D:D + 1], 1.0)

            # --- 1/|q| and 1/|k| in one batch, rsqrt = exp(-0.5*ln(n2)) ---
            # (ln and exp share one act table: no mid-stage table swap)
            sqq = stg_pool.tile([128, 4, D], F32)
            nc.scalar.activation(sqq, q_ps, AF.Square)
            sqk = stg_pool.tile([64, D], F32)
            nc.scalar.activation(sqk, kv_ps[:, 0, :], AF.Square)
            n2all = stg_pool.tile([128, 5], F32)
            nc.gpsimd.memset(n2all[64:128, 4:5], 1.0)
            nc.vector.reduce_sum(n2all[:, 0:4], sqq, axis=mybir.AxisListType.X)
            nc.vector.reduce_sum(n2all[0:64, 4:5], sqk,
                                 axis=mybir.AxisListType.X)
            lnn = stg_pool.tile([128, 5], F32)
            nc.scalar.activation(lnn, n2all, AF.Ln)
            y0 = stg_pool.tile([128, 5], F32)
            nc.scalar.activation(y0, lnn, AF.Exp, scale=-0.5)
            t1 = stg_pool.tile([128, 5], F32)
            nc.vector.tensor_mul(t1, y0, y0)
            nc.vector.tensor_mul(t1, t1, n2all)
            nc.vector.tensor_scalar(t1, t1, -0.5, 1.5, ALU.mult, ALU.add)
            nc.vector.tensor_mul(t1, t1, y0)

            qn = stg_pool.tile([128, 4, D], BF16)
            nc.vector.tensor_tensor(qn, q_ps,
                                    t1[:, 0:4, None].to_broadcast([128, 4, D]),
                                    ALU.mult)
            kn = stg_pool.tile([64, D], BF16)
            nc.vector.tensor_tensor(kn, kv_ps[:, 0, :],
                                    t1[0:64, 4:5].to_broadcast([64, D]),
                                    ALU.mult)

            qnT_ps = psmm.tile([64, 4, 128], BF16, tag="op")
            for j in range(4):
                nc.tensor.transpose(qnT_ps[:, j, :], qn[:, j, :], idbf)
            qnT_sb = stg_pool.tile([64, 4, 128], BF16)
            nc.scalar.copy(qnT_sb, qnT_ps)
            knT_ps = psmm.tile([64, 64], BF16, tag="op")
            nc.tensor.transpose(knT_ps, kn, idbf[0:64, 0:64])
            knT_sb = stg_pool.tile([64, D], BF16)
            nc.vector.tensor_copy(knT_sb, knT_ps)

            # transposed logits only; the av matmul's ones column gives the
            # softmax denominator
            LT_ps = psmm.tile([64, 4, 128], F32, tag="fx")
            for j in range(4):
                nc.tensor.matmul(LT_ps[:, j, :], knT_sb, qnT_sb[:, j, :],
                                 start=(j == 0), stop=(j == 3))
            eT_sb = stg_pool.tile([64, 4, 128], BF16)
            nc.scalar.activation(eT_sb, LT_ps, AF.Exp, scale=attn_scale)

            av_ps = psmm.tile([128, 4, D + 1], F32, tag="lg")
            for j in range(4):
                nc.tensor.matmul(av_ps[:, j, :], eT_sb[:, j, :], v_sb,
                                 start=(j == 0), stop=(j == 3))
            ra = stg_pool.tile([128, 4], F32)
            nc.vector.reciprocal(ra, av_ps[:, :, D])

            os_sb = stg_pool.tile([128, 4, D], BF16)
            nc.vector.tensor_tensor(os_sb, av_ps[:, :, 0:D],
                                    ra[:, :, None].to_broadcast([128, 4, D]),
                                    ALU.mult)
            nc.vector.scalar_tensor_tensor(os_sb, st_sb, res_scale, os_sb,
                                           ALU.mult, ALU.add)

            osT_ps = psmm.tile([64, 4, 128], BF16, tag="op")
            for j in range(4):
                nc.tensor.transpose(osT_ps[:, j, :], os_sb[:, j, :], idbf)
            osT_sb = stg_pool.tile([64, 4, 128], BF16)
            nc.scalar.copy(osT_sb, osT_ps)

            # C[hg, f] in chunk-pair layout matching swT_store chunks
            C_psA = psmm.tile([128, 2, DIM], F32, tag="lg")
            C_psB = psmm.tile([128, 2, DIM], F32, tag="fx")
            for ci in range(4):
                C_half = C_psA if ci < 2 else C_psB
                for par in range(2):
                    nc.tensor.matmul(
                        C_half[64 * par:64 * par + 64, ci % 2, :],
                        osT_sb[:, ci, ts(par, 64)],
                        WoT_sb[:, 2 * ci + par, :],
                        start=True, stop=True)
            C_sb = stg_pool.tile([128, 4, DIM], BF16)
            nc.vector.tensor_copy(C_sb[:, 0:2, :], C_psA)
            nc.scalar.copy(C_sb[:, 2:4, :], C_psB)

            # ================= PASS 2 (transposed output) =================
            # out^T[f, tok] = sum_cc C[cc]^T . swT[cc] — C chunk stationary,
            # 4 tiles of swT as a 512-wide moving operand
            for q in range(T // 4):
                for fh in range(2):
                    # rotate across all three 1-bank PSUM tags: 6 output
                    # tiles in flight, so matmuls never wait on the copies
                    otag = ("op", "lg", "fx")[(2 * q + fh) % 3]
                    oT = psmm.tile([128, 4, TOK], F32, tag=otag,
                                   name=f"oT{fh}_{q}")
                    for cc in range(4):
                        nc.tensor.matmul(
                            oT, C_sb[:, cc, ts(fh, 128)],
                            swT_store[:, 4 * q:4 * q + 4, cc, :],
                            start=(cc == 0), stop=(cc == 3))
                    ob = obufp.tile([128, 4, TOK], BF16, tag=f"obT{fh}",
                                    name=f"ob{fh}_{q}")
                    if fh == 0:
                        nc.vector.tensor_copy(ob, oT)
                    else:
                        nc.scalar.copy(ob, oT)
                    nc.sync.dma_start(
                        out_d[fh, :, 4 * q * TOK:4 * (q + 1) * TOK],
                        ob.rearrange("p a b -> p (a b)"))

    nc.finalize()
    return nc


def kernel(x, Wfx, bfx, Wx, bx, Wslice, bslice, temp, Wq, Wk, Wv,
           res_scale, attn_scale, Wout, bout):
    x = np.asarray(x, dtype=np.float32)
    Wfx = np.asarray(Wfx, np.float32); bfx = np.asarray(bfx, np.float32)
    Wx = np.asarray(Wx, np.float32); bx = np.asarray(bx, np.float32)
    Wslice = np.asarray(Wslice, np.float32); bslice = np.asarray(bslice, np.float32)
    temp = np.asarray(temp, np.float32).reshape(H)
    Wq = np.asarray(Wq, np.float32); Wk = np.asarray(Wk, np.float32)
    Wv = np.asarray(Wv, np.float32)
    res_scale_f = float(np.asarray(res_scale, np.float32))
    attn = np.asarray(attn_scale, np.float32).reshape(H)
    Wout = np.asarray(Wout, np.float32); bout = np.asarray(bout, np.float32)

    assert np.all(np.abs(bfx) == 0) and np.all(np.abs(bx) == 0) \
        and np.all(np.abs(bslice) == 0), "nonzero projection biases unsupported"
    assert np.ptp(attn) == 0, "non-uniform attn_scale unsupported"
    attn_f = float(attn[0])

    # folded logits weight: logits[:, h*G+g] = x @ ((Wslice @ Wx_h)/temp_h).T
    A = np.concatenate(
        [(Wslice @ Wx[h * D:(h + 1) * D, :]) / temp[h] for h in range(H)], axis=0)
    AWT = np.concatenate([A.T, Wfx.T], axis=1)
    AWT = np.ascontiguousarray(AWT).astype(ml_dtypes.bfloat16)          # [256, 1024]
    WoT = np.ascontiguousarray(Wout.T).astype(ml_dtypes.bfloat16)       # [512, 256]
    WqT = np.ascontiguousarray(Wq.T).astype(ml_dtypes.bfloat16)
    WkT = np.ascontiguousarray(Wk.T / H).astype(ml_dtypes.bfloat16)
    WvT = np.ascontiguousarray(Wv.T / H).astype(ml_dtypes.bfloat16)
    idbf = np.eye(128, dtype=np.float32).astype(ml_dtypes.bfloat16)

    key = (attn_f, res_scale_f)
    if key not in _CACHE:
        _CACHE[key] = _build(attn_f, res_scale_f)
    nc = _CACHE[key]

    in_maps = []
    for c in range(NCORES):
        b, half = c // 2, c % 2
        xs = x[b, half * NLOC:(half + 1) * NLOC, :]       # [16384, 256]
        xT = np.ascontiguousarray(xs.T.astype(ml_dtypes.bfloat16))
        in_maps.append(dict(xT=xT, AWT=AWT, idbf=idbf,
                            WqT=WqT, WkT=WkT, WvT=WvT, WoT=WoT))

    global _LAST_IN_MAPS
    _LAST_IN_MAPS = in_maps
    res = bass_utils.run_bass_kernel_spmd(nc, in_maps, core_ids=list(range(NCORES)))

    out = np.empty((B, N, DIM), np.float32)
    for c in range(NCORES):
        b, half = c // 2, c % 2
        oc = res.results[c]["out"].astype(np.float32)   # [2, 128, NLOC]
        out[b, half * NLOC:(half + 1) * NLOC, :] = \
            oc.reshape(DIM, NLOC).T
    if np.any(bout):
        out += bout
    return out
